# revision 32
# baseline (speedup 1.0000x reference)
"""Causal self-attention (B=4, T=2048, D=1024, H=16, DH=64) on 8 TRN2 NeuronCores.

Sharding: core c handles batch b = c//2 and head group hg = (c%2)*8 (8 of 16
heads), Megatron-style on the head dim. Each core computes QKV for its heads,
causal attention, and its partial output projection; the host sums the two
partial projections per batch.

On-chip layout (per core):
  - bf16 datapath for x/wqk/wv/q/k/v/p/masks (halves DMA + DVE cost); the
    projection path (yt, wp) and the softmax normalizer stay f32r/f32.
  - wqk is fully resident in SBUF (loaded once); only x streams per slab.
  - qkv computed transposed: q^T/k^T as [feat(128-part), tok] tiles, v in
    natural [tok, feat] layout with a LEADING ones column so the PV matmul
    also produces the softmax normalizer l at PSUM partition 0.
  - softmax without max-subtraction (scores ~ N(0,1): exp never overflows);
    head PAIRS share one [128,1024] score PSUM tile and a single exp
    activation (the scalar queue is the pipeline bottleneck); causal masking
    by multiplying exp tiles with 0/1 masks on diagonal blocks.
  - all attention matmuls in bf16 (full PE rate); projection in float32r.
"""
import sys
import types

import numpy as np

# If the image lacks antenv.axon_hooks, register a compatible stub so
# run_bass_kernel_spmd(trace=True)/BASS_TRACE=1 can capture NTFF profiles
# (falls back to no-op when the axon client library has no profile export).
try:
    import antenv.axon_hooks  # noqa: F401
except ImportError:
    try:
        from trn_agent_boot.trn_boot import _ntff_profile_via_ctypes

        _hook = _ntff_profile_via_ctypes("/opt/axon/libaxon_pjrt.so")
    except Exception:
        _hook = None
    _m = types.ModuleType("antenv.axon_hooks")
    _m.get_axon_ntff_profile_hook = lambda: _hook
    _m.set_axon_ntff_profile_hook = lambda h: None
    sys.modules["antenv.axon_hooks"] = _m

import ml_dtypes

import concourse.bass_utils as _bass_utils

if getattr(_bass_utils, "_local_artifacts_patch", None) is None:
    _bass_utils.upload_artifacts = lambda tmpdir: tmpdir
    _bass_utils._local_artifacts_patch = True

import concourse.bacc as bacc
import concourse.tile as tile
from concourse import mybir
from concourse.bass_utils import run_bass_kernel_spmd

F32 = mybir.dt.float32
F32R = mybir.dt.float32r
BF16 = mybir.dt.bfloat16
EXP = mybir.ActivationFunctionType.Exp
NP_BF16 = ml_dtypes.bfloat16

B, T, D = 4, 2048, 1024
H, DH = 16, 64
HPC = 8             # heads per core
P = 128
NSLAB = T // 512    # 4 query slabs
DC = D // P         # 8 d-chunks
N_CORES = 8

# f-chunk emission order: q/k for head-pair 0 first, then pair 1, ...
F_ORDER = (0, 4, 1, 5, 2, 6, 3, 7)

_cached_nc = None
LAST_EXEC_NS = None


def _build_program():
    nc = bacc.Bacc("TRN2", target_bir_lowering=False, debug=False, num_devices=N_CORES)
    # all inputs pre-arranged on host to partition-major layouts (contiguous
    # per-partition DMA runs)
    xt_d = nc.dram_tensor("xt", [P, DC, T], BF16, kind="ExternalInput").ap()
    wqk_d = nc.dram_tensor("wqk", [8, P, DC, P], BF16, kind="ExternalInput").ap()
    wv_d = nc.dram_tensor("wv", [P, DC, HPC * DH], BF16, kind="ExternalInput").ap()
    wp_d = nc.dram_tensor("wp", [P, HPC * DH // P, D], F32R, kind="ExternalInput").ap()
    masks_d = nc.dram_tensor("masks", [P, 384], BF16, kind="ExternalInput").ap()
    out_d = nc.dram_tensor("out", [T, D], F32, kind="ExternalOutput").ap()

    with tile.TileContext(nc) as tc:
        lp = nc.allow_low_precision(reason="bf16/fp32r matmul inputs")
        lp.__enter__()
        with (
            tc.tile_pool(name="persist", bufs=1) as persist,
            tc.tile_pool(name="small", bufs=1) as small,
            tc.tile_pool(name="xs", bufs=2) as xpool,
            tc.tile_pool(name="wvs", bufs=1) as wvpool,
            tc.tile_pool(name="wp2", bufs=1) as wppool,
            tc.tile_pool(name="yt", bufs=1) as ytpool,
            tc.tile_pool(name="pp", bufs=6) as ppool,
            tc.tile_pool(name="tails", bufs=2) as tails,
            tc.tile_pool(name="outsb", bufs=3) as outsb,
            tc.tile_pool(name="qkps", bufs=2, space="PSUM") as qkps,
            tc.tile_pool(name="sps", bufs=2, space="PSUM") as sps,
            tc.tile_pool(name="pvps", bufs=2, space="PSUM") as pvps,
        ):
            # masks[:, 0:128] = within-tile triangle (q_local >= k_local);
            # masks[:, 128:384] = p=3 tail: slab cols 256..512 (zeros then triangle)
            masks = persist.tile([P, 384], BF16)
            # k^T persistent feature tiles; q^T lives in a 2-slab ring (a slab's
            # q is only read by its own attention pass), zero-padded per head to
            # 128 partitions so the scores matmul contracts K=128 (the other
            # head's k rows meet zeros)
            qk_k = persist.tile([P, 4, T], BF16)
            qp = persist.tile([P, HPC, 2, 512], BF16)
            nc.gpsimd.memset(qp, 0.0)
            # v natural layout + leading ones column: [tok-tile, head, 1+dh]
            # (ones first so the PV matmul puts l at PSUM partition 0, where
            # partition_broadcast can read it directly)
            vt = persist.tile([P, T // P, HPC, DH + 1], BF16)
            ones_f = small.tile([P, (T // P) * HPC], BF16)
            nc.gpsimd.memset(ones_f, 1.0)
            nc.gpsimd.tensor_copy(
                vt[:, :, :, 0:1],
                ones_f.rearrange("p (a b) -> p a b", a=T // P).unsqueeze(3),
            )
            # y^T: one ring per slab (proj fillers are all deferred to slab 3
            # to balance the ACT-bound late slabs); chunk c rows 0..63 head 2c,
            # 64..127 head 2c+1
            yt = ytpool.tile([P, HPC // 2, NSLAB, 512], F32R)
            wp = wppool.tile([P, HPC * DH // P, D], F32R)
            wv_s = wvpool.tile([P, DC, 512], BF16)
            # qkv weights fully resident: [f-chunk, d-chunk, 128 out-feats]
            wqk_s = persist.tile([P, 8, DC, P], BF16)

            def qk_copy(f, jslab, ps):
                if f < 4:
                    nc.vector.tensor_copy(qp[0:64, 2 * f, jslab % 2, :], ps[0:64, :])
                    nc.vector.tensor_copy(qp[64:128, 2 * f + 1, jslab % 2, :], ps[64:128, :])
                else:
                    nc.vector.tensor_copy(qk_k[:, f - 4, 512 * jslab : 512 * (jslab + 1)], ps)

            def filler_gen(j, xs_cur, xs_next):
                """Generator emitting one PE filler matmul per next(): slab 0's
                deferred QKV chains, the next slab's QKV; ALL projection chains
                are deferred to slab 3 (the most ACT-bound slab)."""
                if j == 0:
                    nc.scalar.dma_start(wp, wp_d)
                    # rest of slab-0 QKV (prologue emitted f=0,4 and V);
                    # wqk chunks stream just-in-time (prefetch distance 2) so
                    # they don't steal HBM bandwidth from the prologue loads
                    C = (1, 5, 2, 6, 3, 7)
                    nc.scalar.dma_start(wqk_s[:, C[0]], wqk_d[C[0]])
                    nc.scalar.dma_start(wqk_s[:, C[1]], wqk_d[C[1]])
                    for k, f in enumerate(C):
                        if k + 2 < 6:
                            nc.scalar.dma_start(wqk_s[:, C[k + 2]], wqk_d[C[k + 2]])
                        ps = qkps.tile([P, 512], F32, tag="qk")
                        for c in range(DC):
                            nc.tensor.matmul(
                                ps, wqk_s[:, f, c, :], xs_cur[:, c, :],
                                start=(c == 0), stop=(c == DC - 1),
                            )
                            yield
                        qk_copy(f, 0, ps)
                if j + 1 < NSLAB:
                    jn = j + 1
                    # order: q/k of head-pair 0 first, then V (its copies must
                    # land before the next slab's first PV iterations), then
                    # the remaining head pairs
                    for f in (0, 4):
                        ps = qkps.tile([P, 512], F32, tag="qk")
                        for c in range(DC):
                            nc.tensor.matmul(
                                ps, wqk_s[:, f, c, :], xs_next[:, c, :],
                                start=(c == 0), stop=(c == DC - 1),
                            )
                            yield
                        qk_copy(f, jn, ps)
                    for tt in range(4):
                        psv = qkps.tile([P, 512], F32, tag="qk")
                        for c in range(DC):
                            nc.tensor.matmul(
                                psv, xs_next[:, c, P * tt : P * (tt + 1)], wv_s[:, c, :],
                                start=(c == 0), stop=(c == DC - 1),
                            )
                            yield
                        nc.vector.tensor_copy(
                            vt[:, 4 * jn + tt, :, 1 : DH + 1],
                            psv.rearrange("p (h d) -> p h d", h=HPC),
                        )
                    for f in (1, 5, 2, 6, 3, 7):
                        ps = qkps.tile([P, 512], F32, tag="qk")
                        for c in range(DC):
                            nc.tensor.matmul(
                                ps, wqk_s[:, f, c, :], xs_next[:, c, :],
                                start=(c == 0), stop=(c == DC - 1),
                            )
                            yield
                        qk_copy(f, jn, ps)
                if j == NSLAB - 1:
                    for jp in range(NSLAB - 1):
                        for lt in range(4):
                            tt = 4 * jp + lt
                            ob = outsb.tile([P, 1024], F32, tag="ob")
                            for e in range(2):
                                pp = qkps.tile([P, 512], F32, tag="qk")
                                for c in range(HPC * DH // P):
                                    nc.tensor.matmul(
                                        pp,
                                        yt[:, c, jp, P * lt : P * (lt + 1)],
                                        wp[:, c, 512 * e : 512 * (e + 1)],
                                        start=(c == 0),
                                        stop=(c == HPC * DH // P - 1),
                                    )
                                    yield
                                nc.vector.tensor_copy(ob[:, 512 * e : 512 * (e + 1)], pp)
                            nc.gpsimd.dma_start(out_d[P * tt : P * (tt + 1), :], ob)

            def emit_proj_direct(j):
                r = j
                for lt in range(4):
                    tt = 4 * j + lt
                    ob = outsb.tile([P, 1024], F32, tag="ob")
                    for e in range(2):
                        pp = qkps.tile([P, 512], F32, tag="qk")
                        for c in range(HPC * DH // P):
                            nc.tensor.matmul(
                                pp,
                                yt[:, c, r, P * lt : P * (lt + 1)],
                                wp[:, c, 512 * e : 512 * (e + 1)],
                                start=(c == 0),
                                stop=(c == HPC * DH // P - 1),
                            )
                        nc.vector.tensor_copy(ob[:, 512 * e : 512 * (e + 1)], pp)
                    nc.gpsimd.dma_start(out_d[P * tt : P * (tt + 1), :], ob)

            # per diagonal position p: column offset the tile is computed from
            C0 = (0, 128, 256, 256)

            def attn_pair(j, hp, fill):
                """Process head pair (2hp, 2hp+1) together: both share the same
                k feature tile (lhsT), their scores fill the two halves of one
                [128,1024] PSUM tile, and ONE activation exps both — halving
                the scalar-queue instruction count (the bottleneck)."""
                r = j          # yt ring (one per slab)
                qr = j % 2     # q tile ring
                kmax = 4 * j + 4
                h0, h1 = 2 * hp, 2 * hp + 1
                pv0 = pvps.tile([P, 512], F32, tag="pv")
                pv1 = pvps.tile([P, 512], F32, tag="pv")

                def c0_of(i):
                    return C0[i - 4 * j] if i >= 4 * j else 0

                p_tiles = {}

                def emit_s(i):
                    c0 = c0_of(i)
                    s_ps = sps.tile([P, 1024], F32, tag="s")
                    kt = qk_k[:, hp, P * i : P * (i + 1)]
                    nc.tensor.matmul(
                        s_ps[:, c0:512], kt, qp[:, h0, qr, c0:512], start=True, stop=True
                    )
                    nc.tensor.matmul(
                        s_ps[:, 512 + c0 : 1024], kt, qp[:, h1, qr, c0:512],
                        start=True, stop=True,
                    )
                    p_sb = ppool.tile([P, 1024], BF16, tag="p")
                    # single exp over both heads' halves; the dead zone
                    # [512, 512+c0) holds stale-but-finite scores, never read
                    nc.scalar.activation(
                        p_sb[:, c0:1024], s_ps[:, c0:1024], EXP, scale=1.0 / 8.0
                    )
                    if i >= 4 * j:
                        p = i - 4 * j
                        for base in (0, 512):
                            if p == 3:
                                nc.vector.tensor_mul(
                                    p_sb[:, base + 256 : base + 512],
                                    p_sb[:, base + 256 : base + 512],
                                    masks[:, 128:384],
                                )
                            else:
                                nc.vector.tensor_mul(
                                    p_sb[:, base + P * p : base + P * (p + 1)],
                                    p_sb[:, base + P * p : base + P * (p + 1)],
                                    masks[:, 0:128],
                                )
                    p_tiles[i] = p_sb

                def emit_pv(i):
                    c0 = c0_of(i)
                    pt = p_tiles.pop(i)
                    nc.tensor.matmul(
                        pv0[0:65, c0:512], vt[:, i, h0, :], pt[:, c0:512],
                        start=(i == 0), stop=(i == kmax - 1),
                    )
                    nc.tensor.matmul(
                        pv1[0:65, c0:512], vt[:, i, h1, :], pt[:, 512 + c0 : 1024],
                        start=(i == 0), stop=(i == kmax - 1),
                    )

                SKEW = 3
                for i in range(kmax + SKEW):
                    if i < kmax:
                        emit_s(i)
                    if i >= SKEW:
                        emit_pv(i - SKEW)
                    fill()
                # stash l and unnormalized y^T; broadcast l, fast-reciprocal,
                # multiply — no SBUF<->SBUF DMA round trips
                for h, pv in ((h0, pv0), (h1, pv1)):
                    qf = h // 2
                    ytmp = tails.tile([65, 512], F32R, tag="ytmp")
                    nc.vector.tensor_copy(ytmp, pv[0:65, :])
                    if h % 2 == 0:
                        nc.gpsimd.dma_start(yt[0:64, qf, r, :], ytmp[1:65, :])
                    else:
                        nc.gpsimd.dma_start(yt[64:128, qf, r, :], ytmp[1:65, :])
                    rb = tails.tile([P, 512], F32, tag="rb")
                    nc.gpsimd.partition_broadcast(rb, ytmp[0:1, :].bitcast(F32), channels=P)
                    nc.vector.reciprocal_approx_fast(rb, rb)
                    if h % 2 == 0:
                        nc.vector.tensor_mul(
                            yt[0:64, qf, r, :], yt[0:64, qf, r, :], rb[0:64, :]
                        )
                    else:
                        nc.vector.tensor_mul(
                            yt[64:128, qf, r, :], yt[64:128, qf, r, :], rb[64:128, :]
                        )

            # ---- pipelined emission ----
            # prologue: f=0 (q heads 0/1), f=4 (k heads 0/1) and V of slab 0 —
            # just enough for the first attention pair; the other 6 QKV chains
            # are slab-0 fillers.  Critical loads lead the sync queue, the
            # rest issue from the scalar HW-DGE queue.
            xs0 = xpool.tile([P, DC, 512], BF16, tag="xs")
            nc.sync.dma_start(wqk_s[:, 0], wqk_d[0])
            nc.sync.dma_start(xs0[:, 0:4, :], xt_d[:, 0:4, 0:512])
            nc.sync.dma_start(xs0[:, 4:8, :], xt_d[:, 4:8, 0:512])
            nc.sync.dma_start(wqk_s[:, 4], wqk_d[4])
            nc.sync.dma_start(wv_s, wv_d)
            nc.sync.dma_start(masks, masks_d)
            for f in (0, 4):
                ps = qkps.tile([P, 512], F32, tag="qk")
                for c in range(DC):
                    nc.tensor.matmul(
                        ps, wqk_s[:, f, c, :], xs0[:, c, :], start=(c == 0), stop=(c == DC - 1)
                    )
                qk_copy(f, 0, ps)
            for tt in range(4):
                psv = qkps.tile([P, 512], F32, tag="qk")
                for c in range(DC):
                    nc.tensor.matmul(
                        psv, xs0[:, c, P * tt : P * (tt + 1)], wv_s[:, c, :],
                        start=(c == 0), stop=(c == DC - 1),
                    )
                nc.vector.tensor_copy(
                    vt[:, tt, :, 1 : DH + 1], psv.rearrange("p (h d) -> p h d", h=HPC)
                )

            # (steps, effective-iters for pacing, startup delay)
            PACE = {0: (144, 20, 0), 1: (96, 30, 3), 2: (96, 42, 3), 3: (96, 56, 2)}
            xs_cur = xs0
            for j in range(NSLAB):
                xs_next = None
                if j + 1 < NSLAB:
                    xs_next = xpool.tile([P, DC, 512], BF16, tag="xs")
                    nc.gpsimd.dma_start(
                        xs_next, xt_d[:, :, 512 * (j + 1) : 512 * (j + 2)]
                    )
                gen = filler_gen(j, xs_cur, xs_next)
                steps, iters, delay = PACE[j]
                state = [-delay, 0]

                def fill():
                    state[0] += 1
                    if state[0] <= 0:
                        return
                    want = min(steps, steps * state[0] // iters) - state[1]
                    for _ in range(want):
                        if next(gen, "END") == "END":
                            break
                        state[1] += 1

                for hp in range(HPC // 2):
                    attn_pair(j, hp, fill)
                while next(gen, "END") != "END":
                    pass
                if xs_next is not None:
                    xs_cur = xs_next
            emit_proj_direct(NSLAB - 1)
        lp.__exit__(None, None, None)
    nc.compile()
    return nc


def _host_masks():
    m = np.zeros((P, 384), dtype=np.float32)
    for kl in range(P):
        m[kl, 0:128] = (np.arange(128) >= kl).astype(np.float32)       # triangle
        m[kl, 128:384] = (np.arange(256, 512) >= kl + 384).astype(np.float32)  # p=3 tail
    return m


def kernel(x, w_attn, w_proj):
    global _cached_nc, LAST_EXEC_NS
    x = np.asarray(x, dtype=np.float32)
    w_attn = np.asarray(w_attn, dtype=np.float32)
    w_proj = np.asarray(w_proj, dtype=np.float32)

    if _cached_nc is None:
        _cached_nc = _build_program()
    nc = _cached_nc

    masks = _host_masks().astype(NP_BF16)
    in_maps = []
    for c in range(N_CORES):
        b, hg = c // 2, (c % 2) * HPC
        w_q = w_attn[hg * DH : hg * DH + HPC * DH, :]
        w_k = w_attn[D + hg * DH : D + hg * DH + HPC * DH, :]
        w_v = w_attn[2 * D + hg * DH : 2 * D + hg * DH + HPC * DH, :]
        xt = x[b].T  # [D, T]
        wqk = np.concatenate([w_q, w_k], axis=0).T  # [D, 2*HPC*DH]
        wv = w_v.T  # [D, HPC*DH]
        wp = w_proj[:, hg * DH : hg * DH + HPC * DH].T  # [HPC*DH, D]
        in_maps.append(
            {
                # partition-major layouts: [p, c, ...] with d = c*128 + p
                "xt": np.ascontiguousarray(
                    xt.reshape(DC, P, T).transpose(1, 0, 2)
                ).astype(NP_BF16),
                "wqk": np.ascontiguousarray(
                    wqk.reshape(DC, P, 8, P).transpose(2, 1, 0, 3)
                ).astype(NP_BF16),
                "wv": np.ascontiguousarray(
                    wv.reshape(DC, P, HPC * DH).transpose(1, 0, 2)
                ).astype(NP_BF16),
                "wp": np.ascontiguousarray(
                    wp.reshape(HPC * DH // P, P, D).transpose(1, 0, 2)
                ),
                "masks": masks,
            }
        )

    res = run_bass_kernel_spmd(nc, in_maps, list(range(N_CORES)))
    LAST_EXEC_NS = res.exec_time_ns
    y = np.empty((B, T, D), dtype=np.float32)
    for b in range(B):
        y[b] = res.results[2 * b]["out"] + res.results[2 * b + 1]["out"]
    return y


# revision 33
# speedup vs baseline: 1.0017x; 1.0017x over previous
"""Causal self-attention (B=4, T=2048, D=1024, H=16, DH=64) on 8 TRN2 NeuronCores.

Sharding: core c handles batch b = c//2 and head group hg = (c%2)*8 (8 of 16
heads), Megatron-style on the head dim. Each core computes QKV for its heads,
causal attention, and its partial output projection; the host sums the two
partial projections per batch.

On-chip layout (per core):
  - bf16 datapath for x/wqk/wv/q/k/v/p/masks (halves DMA + DVE cost); the
    projection path (yt, wp) and the softmax normalizer stay f32r/f32.
  - wqk is fully resident in SBUF (loaded once); only x streams per slab.
  - qkv computed transposed: q^T/k^T as [feat(128-part), tok] tiles, v in
    natural [tok, feat] layout with a LEADING ones column so the PV matmul
    also produces the softmax normalizer l at PSUM partition 0.
  - softmax without max-subtraction (scores ~ N(0,1): exp never overflows);
    head PAIRS share one [128,1024] score PSUM tile and a single exp
    activation (the scalar queue is the pipeline bottleneck); causal masking
    by multiplying exp tiles with 0/1 masks on diagonal blocks.
  - all attention matmuls in bf16 (full PE rate); projection in float32r.
"""
import sys
import types

import numpy as np

# If the image lacks antenv.axon_hooks, register a compatible stub so
# run_bass_kernel_spmd(trace=True)/BASS_TRACE=1 can capture NTFF profiles
# (falls back to no-op when the axon client library has no profile export).
try:
    import antenv.axon_hooks  # noqa: F401
except ImportError:
    try:
        from trn_agent_boot.trn_boot import _ntff_profile_via_ctypes

        _hook = _ntff_profile_via_ctypes("/opt/axon/libaxon_pjrt.so")
    except Exception:
        _hook = None
    _m = types.ModuleType("antenv.axon_hooks")
    _m.get_axon_ntff_profile_hook = lambda: _hook
    _m.set_axon_ntff_profile_hook = lambda h: None
    sys.modules["antenv.axon_hooks"] = _m

import ml_dtypes

import concourse.bass_utils as _bass_utils

if getattr(_bass_utils, "_local_artifacts_patch", None) is None:
    _bass_utils.upload_artifacts = lambda tmpdir: tmpdir
    _bass_utils._local_artifacts_patch = True

import concourse.bacc as bacc
import concourse.tile as tile
from concourse import mybir
from concourse.bass_utils import run_bass_kernel_spmd

F32 = mybir.dt.float32
F32R = mybir.dt.float32r
BF16 = mybir.dt.bfloat16
EXP = mybir.ActivationFunctionType.Exp
NP_BF16 = ml_dtypes.bfloat16

B, T, D = 4, 2048, 1024
H, DH = 16, 64
HPC = 8             # heads per core
P = 128
NSLAB = T // 512    # 4 query slabs
DC = D // P         # 8 d-chunks
N_CORES = 8

# f-chunk emission order: q/k for head-pair 0 first, then pair 1, ...
F_ORDER = (0, 4, 1, 5, 2, 6, 3, 7)

_cached_nc = None
LAST_EXEC_NS = None


def _build_program():
    nc = bacc.Bacc("TRN2", target_bir_lowering=False, debug=False, num_devices=N_CORES)
    # all inputs pre-arranged on host to partition-major layouts (contiguous
    # per-partition DMA runs)
    xt_d = nc.dram_tensor("xt", [P, DC, T], BF16, kind="ExternalInput").ap()
    wqk_d = nc.dram_tensor("wqk", [8, P, DC, P], BF16, kind="ExternalInput").ap()
    wv_d = nc.dram_tensor("wv", [P, DC, HPC * DH], BF16, kind="ExternalInput").ap()
    wp_d = nc.dram_tensor("wp", [P, HPC * DH // P, D], F32R, kind="ExternalInput").ap()
    masks_d = nc.dram_tensor("masks", [P, 384], BF16, kind="ExternalInput").ap()
    out_d = nc.dram_tensor("out", [T, D], F32, kind="ExternalOutput").ap()

    with tile.TileContext(nc) as tc:
        lp = nc.allow_low_precision(reason="bf16/fp32r matmul inputs")
        lp.__enter__()
        with (
            tc.tile_pool(name="persist", bufs=1) as persist,
            tc.tile_pool(name="small", bufs=1) as small,
            tc.tile_pool(name="xs", bufs=2) as xpool,
            tc.tile_pool(name="wvs", bufs=1) as wvpool,
            tc.tile_pool(name="wp2", bufs=1) as wppool,
            tc.tile_pool(name="yt", bufs=1) as ytpool,
            tc.tile_pool(name="pp", bufs=6) as ppool,
            tc.tile_pool(name="tails", bufs=2) as tails,
            tc.tile_pool(name="outsb", bufs=3) as outsb,
            tc.tile_pool(name="qkps", bufs=2, space="PSUM") as qkps,
            tc.tile_pool(name="sps", bufs=2, space="PSUM") as sps,
            tc.tile_pool(name="pvps", bufs=2, space="PSUM") as pvps,
        ):
            # masks[:, 0:128] = within-tile triangle (q_local >= k_local);
            # masks[:, 128:384] = p=3 tail: slab cols 256..512 (zeros then triangle)
            masks = persist.tile([P, 384], BF16)
            # k^T persistent feature tiles; q^T lives in a 2-slab ring (a slab's
            # q is only read by its own attention pass), zero-padded per head to
            # 128 partitions so the scores matmul contracts K=128 (the other
            # head's k rows meet zeros)
            qk_k = persist.tile([P, 4, T], BF16)
            qp = persist.tile([P, HPC, 2, 512], BF16)
            nc.gpsimd.memset(qp, 0.0)
            # v natural layout + leading ones column: [tok-tile, head, 1+dh]
            # (ones first so the PV matmul puts l at PSUM partition 0, where
            # partition_broadcast can read it directly)
            vt = persist.tile([P, T // P, HPC, DH + 1], BF16)
            ones_f = small.tile([P, (T // P) * HPC], BF16)
            nc.gpsimd.memset(ones_f, 1.0)
            nc.gpsimd.tensor_copy(
                vt[:, :, :, 0:1],
                ones_f.rearrange("p (a b) -> p a b", a=T // P).unsqueeze(3),
            )
            # y^T: one ring per slab (proj fillers are all deferred to slab 3
            # to balance the ACT-bound late slabs); chunk c rows 0..63 head 2c,
            # 64..127 head 2c+1
            yt = ytpool.tile([P, HPC // 2, NSLAB, 512], F32R)
            wp = wppool.tile([P, HPC * DH // P, D], F32R)
            wv_s = wvpool.tile([P, DC, 512], BF16)
            # qkv weights fully resident: [f-chunk, d-chunk, 128 out-feats]
            wqk_s = persist.tile([P, 8, DC, P], BF16)

            def qk_copy(f, jslab, ps):
                if f < 4:
                    nc.vector.tensor_copy(qp[0:64, 2 * f, jslab % 2, :], ps[0:64, :])
                    nc.vector.tensor_copy(qp[64:128, 2 * f + 1, jslab % 2, :], ps[64:128, :])
                else:
                    nc.vector.tensor_copy(qk_k[:, f - 4, 512 * jslab : 512 * (jslab + 1)], ps)

            def filler_gen(j, xs_cur, xs_next):
                """Generator emitting one PE filler matmul per next(): slab 0's
                deferred QKV chains, the next slab's QKV; ALL projection chains
                are deferred to slab 3 (the most ACT-bound slab)."""
                if j == 0:
                    nc.scalar.dma_start(wp, wp_d)
                    # rest of slab-0 QKV (prologue emitted f=0,4 and V);
                    # wqk chunks stream just-in-time (prefetch distance 2) so
                    # they don't steal HBM bandwidth from the prologue loads
                    C = (1, 5, 2, 6, 3, 7)
                    nc.scalar.dma_start(wqk_s[:, C[0]], wqk_d[C[0]])
                    nc.scalar.dma_start(wqk_s[:, C[1]], wqk_d[C[1]])
                    for k, f in enumerate(C):
                        if k + 2 < 6:
                            nc.scalar.dma_start(wqk_s[:, C[k + 2]], wqk_d[C[k + 2]])
                        ps = qkps.tile([P, 512], F32, tag="qk")
                        for c in range(DC):
                            nc.tensor.matmul(
                                ps, wqk_s[:, f, c, :], xs_cur[:, c, :],
                                start=(c == 0), stop=(c == DC - 1),
                            )
                            yield
                        qk_copy(f, 0, ps)
                if j + 1 < NSLAB:
                    jn = j + 1
                    # order: q/k of head-pair 0 first, then V (its copies must
                    # land before the next slab's first PV iterations), then
                    # the remaining head pairs
                    for f in (0, 4):
                        ps = qkps.tile([P, 512], F32, tag="qk")
                        for c in range(DC):
                            nc.tensor.matmul(
                                ps, wqk_s[:, f, c, :], xs_next[:, c, :],
                                start=(c == 0), stop=(c == DC - 1),
                            )
                            yield
                        qk_copy(f, jn, ps)
                    for tt in range(4):
                        psv = qkps.tile([P, 512], F32, tag="qk")
                        for c in range(DC):
                            nc.tensor.matmul(
                                psv, xs_next[:, c, P * tt : P * (tt + 1)], wv_s[:, c, :],
                                start=(c == 0), stop=(c == DC - 1),
                            )
                            yield
                        nc.vector.tensor_copy(
                            vt[:, 4 * jn + tt, :, 1 : DH + 1],
                            psv.rearrange("p (h d) -> p h d", h=HPC),
                        )
                    for f in (1, 5, 2, 6, 3, 7):
                        ps = qkps.tile([P, 512], F32, tag="qk")
                        for c in range(DC):
                            nc.tensor.matmul(
                                ps, wqk_s[:, f, c, :], xs_next[:, c, :],
                                start=(c == 0), stop=(c == DC - 1),
                            )
                            yield
                        qk_copy(f, jn, ps)
                if j == NSLAB - 1:
                    for jp in range(NSLAB - 1):
                        for lt in range(4):
                            tt = 4 * jp + lt
                            ob = outsb.tile([P, 1024], F32, tag="ob")
                            for e in range(2):
                                pp = qkps.tile([P, 512], F32, tag="qk")
                                for c in range(HPC * DH // P):
                                    nc.tensor.matmul(
                                        pp,
                                        yt[:, c, jp, P * lt : P * (lt + 1)],
                                        wp[:, c, 512 * e : 512 * (e + 1)],
                                        start=(c == 0),
                                        stop=(c == HPC * DH // P - 1),
                                    )
                                    yield
                                nc.vector.tensor_copy(ob[:, 512 * e : 512 * (e + 1)], pp)
                            nc.gpsimd.dma_start(out_d[P * tt : P * (tt + 1), :], ob)

            def emit_proj_direct(j):
                r = j
                for lt in range(4):
                    tt = 4 * j + lt
                    ob = outsb.tile([P, 1024], F32, tag="ob")
                    for e in range(2):
                        pp = qkps.tile([P, 512], F32, tag="qk")
                        for c in range(HPC * DH // P):
                            nc.tensor.matmul(
                                pp,
                                yt[:, c, r, P * lt : P * (lt + 1)],
                                wp[:, c, 512 * e : 512 * (e + 1)],
                                start=(c == 0),
                                stop=(c == HPC * DH // P - 1),
                            )
                        nc.vector.tensor_copy(ob[:, 512 * e : 512 * (e + 1)], pp)
                    nc.gpsimd.dma_start(out_d[P * tt : P * (tt + 1), :], ob)

            # per diagonal position p: column offset the tile is computed from
            C0 = (0, 128, 256, 256)

            def attn_pair(j, hp, fill):
                """Process head pair (2hp, 2hp+1) together: both share the same
                k feature tile (lhsT), their scores fill the two halves of one
                [128,1024] PSUM tile, and ONE activation exps both — halving
                the scalar-queue instruction count (the bottleneck)."""
                r = j          # yt ring (one per slab)
                qr = j % 2     # q tile ring
                kmax = 4 * j + 4
                h0, h1 = 2 * hp, 2 * hp + 1
                pv0 = pvps.tile([P, 512], F32, tag="pv")
                pv1 = pvps.tile([P, 512], F32, tag="pv")

                def c0_of(i):
                    return C0[i - 4 * j] if i >= 4 * j else 0

                p_tiles = {}

                def emit_s(i):
                    c0 = c0_of(i)
                    s_ps = sps.tile([P, 1024], F32, tag="s")
                    kt = qk_k[:, hp, P * i : P * (i + 1)]
                    nc.tensor.matmul(
                        s_ps[:, c0:512], kt, qp[:, h0, qr, c0:512], start=True, stop=True
                    )
                    nc.tensor.matmul(
                        s_ps[:, 512 + c0 : 1024], kt, qp[:, h1, qr, c0:512],
                        start=True, stop=True,
                    )
                    p_sb = ppool.tile([P, 1024], BF16, tag="p")
                    # single exp over both heads' halves; the dead zone
                    # [512, 512+c0) holds stale-but-finite scores, never read
                    nc.scalar.activation(
                        p_sb[:, c0:1024], s_ps[:, c0:1024], EXP, scale=1.0 / 8.0
                    )
                    if i >= 4 * j:
                        p = i - 4 * j
                        for base in (0, 512):
                            if p == 3:
                                nc.vector.tensor_mul(
                                    p_sb[:, base + 256 : base + 512],
                                    p_sb[:, base + 256 : base + 512],
                                    masks[:, 128:384],
                                )
                            else:
                                nc.vector.tensor_mul(
                                    p_sb[:, base + P * p : base + P * (p + 1)],
                                    p_sb[:, base + P * p : base + P * (p + 1)],
                                    masks[:, 0:128],
                                )
                    p_tiles[i] = p_sb

                def emit_pv(i):
                    c0 = c0_of(i)
                    pt = p_tiles.pop(i)
                    nc.tensor.matmul(
                        pv0[0:65, c0:512], vt[:, i, h0, :], pt[:, c0:512],
                        start=(i == 0), stop=(i == kmax - 1),
                    )
                    nc.tensor.matmul(
                        pv1[0:65, c0:512], vt[:, i, h1, :], pt[:, 512 + c0 : 1024],
                        start=(i == 0), stop=(i == kmax - 1),
                    )

                SKEW = 2
                for i in range(kmax + SKEW):
                    if i < kmax:
                        emit_s(i)
                    if i >= SKEW:
                        emit_pv(i - SKEW)
                    fill()
                # stash l and unnormalized y^T; broadcast l, fast-reciprocal,
                # multiply — no SBUF<->SBUF DMA round trips
                for h, pv in ((h0, pv0), (h1, pv1)):
                    qf = h // 2
                    ytmp = tails.tile([65, 512], F32R, tag="ytmp")
                    nc.vector.tensor_copy(ytmp, pv[0:65, :])
                    if h % 2 == 0:
                        nc.gpsimd.dma_start(yt[0:64, qf, r, :], ytmp[1:65, :])
                    else:
                        nc.gpsimd.dma_start(yt[64:128, qf, r, :], ytmp[1:65, :])
                    rb = tails.tile([P, 512], F32, tag="rb")
                    nc.gpsimd.partition_broadcast(rb, ytmp[0:1, :].bitcast(F32), channels=P)
                    nc.vector.reciprocal_approx_fast(rb, rb)
                    if h % 2 == 0:
                        nc.vector.tensor_mul(
                            yt[0:64, qf, r, :], yt[0:64, qf, r, :], rb[0:64, :]
                        )
                    else:
                        nc.vector.tensor_mul(
                            yt[64:128, qf, r, :], yt[64:128, qf, r, :], rb[64:128, :]
                        )

            # ---- pipelined emission ----
            # prologue: f=0 (q heads 0/1), f=4 (k heads 0/1) and V of slab 0 —
            # just enough for the first attention pair; the other 6 QKV chains
            # are slab-0 fillers.  Critical loads lead the sync queue, the
            # rest issue from the scalar HW-DGE queue.
            xs0 = xpool.tile([P, DC, 512], BF16, tag="xs")
            nc.sync.dma_start(wqk_s[:, 0], wqk_d[0])
            nc.sync.dma_start(xs0[:, 0:4, :], xt_d[:, 0:4, 0:512])
            nc.sync.dma_start(xs0[:, 4:8, :], xt_d[:, 4:8, 0:512])
            nc.sync.dma_start(wqk_s[:, 4], wqk_d[4])
            nc.sync.dma_start(wv_s, wv_d)
            nc.sync.dma_start(masks, masks_d)
            for f in (0, 4):
                ps = qkps.tile([P, 512], F32, tag="qk")
                for c in range(DC):
                    nc.tensor.matmul(
                        ps, wqk_s[:, f, c, :], xs0[:, c, :], start=(c == 0), stop=(c == DC - 1)
                    )
                qk_copy(f, 0, ps)
            for tt in range(4):
                psv = qkps.tile([P, 512], F32, tag="qk")
                for c in range(DC):
                    nc.tensor.matmul(
                        psv, xs0[:, c, P * tt : P * (tt + 1)], wv_s[:, c, :],
                        start=(c == 0), stop=(c == DC - 1),
                    )
                nc.vector.tensor_copy(
                    vt[:, tt, :, 1 : DH + 1], psv.rearrange("p (h d) -> p h d", h=HPC)
                )

            # (steps, effective-iters for pacing, startup delay)
            PACE = {0: (144, 20, 0), 1: (96, 30, 3), 2: (96, 42, 3), 3: (96, 56, 2)}
            xs_cur = xs0
            for j in range(NSLAB):
                xs_next = None
                if j + 1 < NSLAB:
                    xs_next = xpool.tile([P, DC, 512], BF16, tag="xs")
                    nc.gpsimd.dma_start(
                        xs_next, xt_d[:, :, 512 * (j + 1) : 512 * (j + 2)]
                    )
                gen = filler_gen(j, xs_cur, xs_next)
                steps, iters, delay = PACE[j]
                state = [-delay, 0]

                def fill():
                    state[0] += 1
                    if state[0] <= 0:
                        return
                    want = min(steps, steps * state[0] // iters) - state[1]
                    for _ in range(want):
                        if next(gen, "END") == "END":
                            break
                        state[1] += 1

                for hp in range(HPC // 2):
                    attn_pair(j, hp, fill)
                while next(gen, "END") != "END":
                    pass
                if xs_next is not None:
                    xs_cur = xs_next
            emit_proj_direct(NSLAB - 1)
        lp.__exit__(None, None, None)
    nc.compile()
    return nc


def _host_masks():
    m = np.zeros((P, 384), dtype=np.float32)
    for kl in range(P):
        m[kl, 0:128] = (np.arange(128) >= kl).astype(np.float32)       # triangle
        m[kl, 128:384] = (np.arange(256, 512) >= kl + 384).astype(np.float32)  # p=3 tail
    return m


def kernel(x, w_attn, w_proj):
    global _cached_nc, LAST_EXEC_NS
    x = np.asarray(x, dtype=np.float32)
    w_attn = np.asarray(w_attn, dtype=np.float32)
    w_proj = np.asarray(w_proj, dtype=np.float32)

    if _cached_nc is None:
        _cached_nc = _build_program()
    nc = _cached_nc

    masks = _host_masks().astype(NP_BF16)
    in_maps = []
    for c in range(N_CORES):
        b, hg = c // 2, (c % 2) * HPC
        w_q = w_attn[hg * DH : hg * DH + HPC * DH, :]
        w_k = w_attn[D + hg * DH : D + hg * DH + HPC * DH, :]
        w_v = w_attn[2 * D + hg * DH : 2 * D + hg * DH + HPC * DH, :]
        xt = x[b].T  # [D, T]
        wqk = np.concatenate([w_q, w_k], axis=0).T  # [D, 2*HPC*DH]
        wv = w_v.T  # [D, HPC*DH]
        wp = w_proj[:, hg * DH : hg * DH + HPC * DH].T  # [HPC*DH, D]
        in_maps.append(
            {
                # partition-major layouts: [p, c, ...] with d = c*128 + p
                "xt": np.ascontiguousarray(
                    xt.reshape(DC, P, T).transpose(1, 0, 2)
                ).astype(NP_BF16),
                "wqk": np.ascontiguousarray(
                    wqk.reshape(DC, P, 8, P).transpose(2, 1, 0, 3)
                ).astype(NP_BF16),
                "wv": np.ascontiguousarray(
                    wv.reshape(DC, P, HPC * DH).transpose(1, 0, 2)
                ).astype(NP_BF16),
                "wp": np.ascontiguousarray(
                    wp.reshape(HPC * DH // P, P, D).transpose(1, 0, 2)
                ),
                "masks": masks,
            }
        )

    res = run_bass_kernel_spmd(nc, in_maps, list(range(N_CORES)))
    LAST_EXEC_NS = res.exec_time_ns
    y = np.empty((B, T, D), dtype=np.float32)
    for b in range(B):
        y[b] = res.results[2 * b]["out"] + res.results[2 * b + 1]["out"]
    return y


# revision 34
# speedup vs baseline: 1.0316x; 1.0298x over previous
"""Causal self-attention (B=4, T=2048, D=1024, H=16, DH=64) on 8 TRN2 NeuronCores.

Sharding: core c handles batch b = c//2 and head group hg = (c%2)*8 (8 of 16
heads), Megatron-style on the head dim. Each core computes QKV for its heads,
causal attention, and its partial output projection; the host sums the two
partial projections per batch.

On-chip layout (per core):
  - bf16 datapath for x/wqk/wv/q/k/v/p/masks (halves DMA + DVE cost); the
    projection path (yt, wp) and the softmax normalizer stay f32r/f32.
  - wqk is fully resident in SBUF (loaded once); only x streams per slab.
  - qkv computed transposed: q^T/k^T as [feat(128-part), tok] tiles, v in
    natural [tok, feat] layout with a LEADING ones column so the PV matmul
    also produces the softmax normalizer l at PSUM partition 0.
  - softmax without max-subtraction (scores ~ N(0,1): exp never overflows);
    head PAIRS share one [128,1024] score PSUM tile and a single exp
    activation (the scalar queue is the pipeline bottleneck); causal masking
    by multiplying exp tiles with 0/1 masks on diagonal blocks.
  - all attention matmuls in bf16 (full PE rate); projection in float32r.
"""
import sys
import types

import numpy as np

# If the image lacks antenv.axon_hooks, register a compatible stub so
# run_bass_kernel_spmd(trace=True)/BASS_TRACE=1 can capture NTFF profiles
# (falls back to no-op when the axon client library has no profile export).
try:
    import antenv.axon_hooks  # noqa: F401
except ImportError:
    try:
        from trn_agent_boot.trn_boot import _ntff_profile_via_ctypes

        _hook = _ntff_profile_via_ctypes("/opt/axon/libaxon_pjrt.so")
    except Exception:
        _hook = None
    _m = types.ModuleType("antenv.axon_hooks")
    _m.get_axon_ntff_profile_hook = lambda: _hook
    _m.set_axon_ntff_profile_hook = lambda h: None
    sys.modules["antenv.axon_hooks"] = _m

import ml_dtypes

import concourse.bass_utils as _bass_utils

if getattr(_bass_utils, "_local_artifacts_patch", None) is None:
    _bass_utils.upload_artifacts = lambda tmpdir: tmpdir
    _bass_utils._local_artifacts_patch = True

import concourse.bacc as bacc
import concourse.tile as tile
from concourse import mybir
from concourse.bass_utils import run_bass_kernel_spmd

F32 = mybir.dt.float32
F32R = mybir.dt.float32r
BF16 = mybir.dt.bfloat16
EXP = mybir.ActivationFunctionType.Exp
NP_BF16 = ml_dtypes.bfloat16

B, T, D = 4, 2048, 1024
H, DH = 16, 64
HPC = 8             # heads per core
P = 128
NSLAB = T // 512    # 4 query slabs
DC = D // P         # 8 d-chunks
N_CORES = 8

# f-chunk emission order: q/k for head-pair 0 first, then pair 1, ...
F_ORDER = (0, 4, 1, 5, 2, 6, 3, 7)

_cached_nc = None
LAST_EXEC_NS = None


def _build_program():
    nc = bacc.Bacc("TRN2", target_bir_lowering=False, debug=False, num_devices=N_CORES)
    # all inputs pre-arranged on host to partition-major layouts (contiguous
    # per-partition DMA runs)
    xt_d = nc.dram_tensor("xt", [P, DC, T], BF16, kind="ExternalInput").ap()
    wqk_d = nc.dram_tensor("wqk", [8, P, DC, P], BF16, kind="ExternalInput").ap()
    wv_d = nc.dram_tensor("wv", [P, DC, HPC * DH], BF16, kind="ExternalInput").ap()
    wp_d = nc.dram_tensor("wp", [P, HPC * DH // P, D], F32R, kind="ExternalInput").ap()
    masks_d = nc.dram_tensor("masks", [P, 384], BF16, kind="ExternalInput").ap()
    out_d = nc.dram_tensor("out", [T, D], F32, kind="ExternalOutput").ap()

    with tile.TileContext(nc) as tc:
        lp = nc.allow_low_precision(reason="bf16/fp32r matmul inputs")
        lp.__enter__()
        with (
            tc.tile_pool(name="persist", bufs=1) as persist,
            tc.tile_pool(name="small", bufs=1) as small,
            tc.tile_pool(name="xs", bufs=2) as xpool,
            tc.tile_pool(name="wvs", bufs=1) as wvpool,
            tc.tile_pool(name="wp2", bufs=1) as wppool,
            tc.tile_pool(name="yt", bufs=1) as ytpool,
            tc.tile_pool(name="pp", bufs=6) as ppool,
            tc.tile_pool(name="tails", bufs=2) as tails,
            tc.tile_pool(name="outsb", bufs=3) as outsb,
            tc.tile_pool(name="qkps", bufs=2, space="PSUM") as qkps,
            tc.tile_pool(name="sps", bufs=2, space="PSUM") as sps,
            tc.tile_pool(name="pvps", bufs=2, space="PSUM") as pvps,
        ):
            # masks[:, 0:128] = within-tile triangle (q_local >= k_local);
            # masks[:, 128:384] = p=3 tail: slab cols 256..512 (zeros then triangle)
            masks = persist.tile([P, 384], BF16)
            # k^T persistent feature tiles; q^T lives in a 2-slab ring (a slab's
            # q is only read by its own attention pass), zero-padded per head to
            # 128 partitions so the scores matmul contracts K=128 (the other
            # head's k rows meet zeros)
            qk_k = persist.tile([P, 4, T], BF16)
            qp = persist.tile([P, HPC, 2, 512], BF16)
            nc.gpsimd.memset(qp, 0.0)
            # v natural layout + leading ones column: [tok-tile, head, 1+dh]
            # (ones first so the PV matmul puts l at PSUM partition 0, where
            # partition_broadcast can read it directly)
            vt = persist.tile([P, T // P, HPC, DH + 1], BF16)
            ones_f = small.tile([P, (T // P) * HPC], BF16)
            nc.gpsimd.memset(ones_f, 1.0)
            nc.gpsimd.tensor_copy(
                vt[:, :, :, 0:1],
                ones_f.rearrange("p (a b) -> p a b", a=T // P).unsqueeze(3),
            )
            # y^T: one ring per slab (proj fillers are all deferred to slab 3
            # to balance the ACT-bound late slabs); chunk c rows 0..63 head 2c,
            # 64..127 head 2c+1
            yt = ytpool.tile([P, HPC // 2, NSLAB, 512], F32R)
            wp = wppool.tile([P, HPC * DH // P, D], F32R)
            wv_s = wvpool.tile([P, DC, 512], BF16)
            # qkv weights fully resident: [f-chunk, d-chunk, 128 out-feats]
            wqk_s = persist.tile([P, 8, DC, P], BF16)

            def qk_copy(f, jslab, ps):
                if f < 4:
                    nc.vector.tensor_copy(qp[0:64, 2 * f, jslab % 2, :], ps[0:64, :])
                    nc.vector.tensor_copy(qp[64:128, 2 * f + 1, jslab % 2, :], ps[64:128, :])
                else:
                    nc.vector.tensor_copy(qk_k[:, f - 4, 512 * jslab : 512 * (jslab + 1)], ps)

            def filler_gen(j, xs_cur, xs_next):
                """Generator emitting one PE filler matmul per next(): slab 0's
                deferred QKV chains, the next slab's QKV; ALL projection chains
                are deferred to slab 3 (the most ACT-bound slab)."""
                if j == 0:
                    nc.scalar.dma_start(wp, wp_d)
                    # rest of slab-0 QKV (prologue emitted f=0,4 and V);
                    # wqk chunks stream just-in-time (prefetch distance 2) so
                    # they don't steal HBM bandwidth from the prologue loads
                    C = (1, 5, 2, 6, 3, 7)
                    nc.scalar.dma_start(wqk_s[:, C[0]], wqk_d[C[0]])
                    nc.scalar.dma_start(wqk_s[:, C[1]], wqk_d[C[1]])
                    for k, f in enumerate(C):
                        if k + 2 < 6:
                            nc.scalar.dma_start(wqk_s[:, C[k + 2]], wqk_d[C[k + 2]])
                        ps = qkps.tile([P, 512], F32, tag="qk")
                        for c in range(DC):
                            nc.tensor.matmul(
                                ps, wqk_s[:, f, c, :], xs_cur[:, c, :],
                                start=(c == 0), stop=(c == DC - 1),
                            )
                            yield
                        qk_copy(f, 0, ps)
                if j + 1 < NSLAB:
                    jn = j + 1
                    # order: q/k of head-pair 0 first, then V (its copies must
                    # land before the next slab's first PV iterations), then
                    # the remaining head pairs
                    for f in (0, 4):
                        ps = qkps.tile([P, 512], F32, tag="qk")
                        for c in range(DC):
                            nc.tensor.matmul(
                                ps, wqk_s[:, f, c, :], xs_next[:, c, :],
                                start=(c == 0), stop=(c == DC - 1),
                            )
                            yield
                        qk_copy(f, jn, ps)
                    for tt in range(4):
                        psv = qkps.tile([P, 512], F32, tag="qk")
                        for c in range(DC):
                            nc.tensor.matmul(
                                psv, xs_next[:, c, P * tt : P * (tt + 1)], wv_s[:, c, :],
                                start=(c == 0), stop=(c == DC - 1),
                            )
                            yield
                        nc.vector.tensor_copy(
                            vt[:, 4 * jn + tt, :, 1 : DH + 1],
                            psv.rearrange("p (h d) -> p h d", h=HPC),
                        )
                    for f in (1, 5, 2, 6, 3, 7):
                        ps = qkps.tile([P, 512], F32, tag="qk")
                        for c in range(DC):
                            nc.tensor.matmul(
                                ps, wqk_s[:, f, c, :], xs_next[:, c, :],
                                start=(c == 0), stop=(c == DC - 1),
                            )
                            yield
                        qk_copy(f, jn, ps)
                if j == NSLAB - 1:
                    for jp in range(NSLAB - 1):
                        for lt in range(4):
                            tt = 4 * jp + lt
                            ob = outsb.tile([P, 1024], F32, tag="ob")
                            for e in range(2):
                                pp = qkps.tile([P, 512], F32, tag="qk")
                                for c in range(HPC * DH // P):
                                    nc.tensor.matmul(
                                        pp,
                                        yt[:, c, jp, P * lt : P * (lt + 1)],
                                        wp[:, c, 512 * e : 512 * (e + 1)],
                                        start=(c == 0),
                                        stop=(c == HPC * DH // P - 1),
                                    )
                                    yield
                                nc.vector.tensor_copy(ob[:, 512 * e : 512 * (e + 1)], pp)
                            nc.gpsimd.dma_start(out_d[P * tt : P * (tt + 1), :], ob)

            def emit_proj_direct(j):
                r = j
                for lt in range(4):
                    tt = 4 * j + lt
                    ob = outsb.tile([P, 1024], F32, tag="ob")
                    for e in range(2):
                        pp = qkps.tile([P, 512], F32, tag="qk")
                        for c in range(HPC * DH // P):
                            nc.tensor.matmul(
                                pp,
                                yt[:, c, r, P * lt : P * (lt + 1)],
                                wp[:, c, 512 * e : 512 * (e + 1)],
                                start=(c == 0),
                                stop=(c == HPC * DH // P - 1),
                            )
                        nc.vector.tensor_copy(ob[:, 512 * e : 512 * (e + 1)], pp)
                    nc.gpsimd.dma_start(out_d[P * tt : P * (tt + 1), :], ob)

            # per diagonal position p: column offset the tile is computed from
            C0 = (0, 128, 256, 256)

            def attn_pair(j, hp, fill):
                """Process head pair (2hp, 2hp+1) together: both share the same
                k feature tile (lhsT), their scores fill the two halves of one
                [128,1024] PSUM tile, and ONE activation exps both — halving
                the scalar-queue instruction count (the bottleneck)."""
                r = j          # yt ring (one per slab)
                qr = j % 2     # q tile ring
                kmax = 4 * j + 4
                h0, h1 = 2 * hp, 2 * hp + 1
                pv0 = pvps.tile([P, 512], F32, tag="pv")
                pv1 = pvps.tile([P, 512], F32, tag="pv")

                def c0_of(i):
                    return C0[i - 4 * j] if i >= 4 * j else 0

                p_tiles = {}

                def emit_s(i):
                    c0 = c0_of(i)
                    s_ps = sps.tile([P, 1024], F32, tag="s")
                    kt = qk_k[:, hp, P * i : P * (i + 1)]
                    nc.tensor.matmul(
                        s_ps[:, c0:512], kt, qp[:, h0, qr, c0:512], start=True, stop=True
                    )
                    nc.tensor.matmul(
                        s_ps[:, 512 + c0 : 1024], kt, qp[:, h1, qr, c0:512],
                        start=True, stop=True,
                    )
                    p_sb = ppool.tile([P, 1024], BF16, tag="p")
                    # single exp over both heads' halves; the dead zone
                    # [512, 512+c0) holds stale-but-finite scores, never read
                    nc.scalar.activation(
                        p_sb[:, c0:1024], s_ps[:, c0:1024], EXP, scale=1.0 / 8.0
                    )
                    if i >= 4 * j:
                        p = i - 4 * j
                        for base in (0, 512):
                            if p == 3:
                                nc.vector.tensor_mul(
                                    p_sb[:, base + 256 : base + 512],
                                    p_sb[:, base + 256 : base + 512],
                                    masks[:, 128:384],
                                )
                            else:
                                nc.vector.tensor_mul(
                                    p_sb[:, base + P * p : base + P * (p + 1)],
                                    p_sb[:, base + P * p : base + P * (p + 1)],
                                    masks[:, 0:128],
                                )
                    p_tiles[i] = p_sb

                def emit_pv(i):
                    c0 = c0_of(i)
                    pt = p_tiles.pop(i)
                    nc.tensor.matmul(
                        pv0[0:65, c0:512], vt[:, i, h0, :], pt[:, c0:512],
                        start=(i == 0), stop=(i == kmax - 1),
                    )
                    nc.tensor.matmul(
                        pv1[0:65, c0:512], vt[:, i, h1, :], pt[:, 512 + c0 : 1024],
                        start=(i == 0), stop=(i == kmax - 1),
                    )

                SKEW = 2
                for i in range(kmax + SKEW):
                    if i < kmax:
                        emit_s(i)
                    if i >= SKEW:
                        emit_pv(i - SKEW)
                    fill()
                # stash l and unnormalized y^T; broadcast l, fast-reciprocal,
                # multiply — no SBUF<->SBUF DMA round trips
                for h, pv in ((h0, pv0), (h1, pv1)):
                    qf = h // 2
                    ytmp = tails.tile([65, 512], F32R, tag="ytmp")
                    nc.vector.tensor_copy(ytmp, pv[0:65, :])
                    if h % 2 == 0:
                        nc.gpsimd.dma_start(yt[0:64, qf, r, :], ytmp[1:65, :])
                    else:
                        nc.gpsimd.dma_start(yt[64:128, qf, r, :], ytmp[1:65, :])
                    rb = tails.tile([P, 512], F32, tag="rb")
                    nc.gpsimd.partition_broadcast(rb, ytmp[0:1, :].bitcast(F32), channels=P)
                    nc.vector.reciprocal_approx_fast(rb, rb)
                    if h % 2 == 0:
                        nc.vector.tensor_mul(
                            yt[0:64, qf, r, :], yt[0:64, qf, r, :], rb[0:64, :]
                        )
                    else:
                        nc.vector.tensor_mul(
                            yt[64:128, qf, r, :], yt[64:128, qf, r, :], rb[64:128, :]
                        )

            # ---- pipelined emission ----
            # prologue: f=0 (q heads 0/1), f=4 (k heads 0/1) and V of slab 0 —
            # just enough for the first attention pair; the other 6 QKV chains
            # are slab-0 fillers.  Critical loads lead the sync queue, the
            # rest issue from the scalar HW-DGE queue.
            xs0 = xpool.tile([P, DC, 512], BF16, tag="xs")
            nc.sync.dma_start(wqk_s[:, 0], wqk_d[0])
            nc.sync.dma_start(xs0[:, 0:4, :], xt_d[:, 0:4, 0:512])
            nc.sync.dma_start(xs0[:, 4:8, :], xt_d[:, 4:8, 0:512])
            nc.sync.dma_start(wqk_s[:, 4], wqk_d[4])
            nc.sync.dma_start(wv_s, wv_d)
            nc.sync.dma_start(masks, masks_d)
            for f in (0, 4):
                ps = qkps.tile([P, 512], F32, tag="qk")
                for c in range(DC):
                    nc.tensor.matmul(
                        ps, wqk_s[:, f, c, :], xs0[:, c, :], start=(c == 0), stop=(c == DC - 1)
                    )
                qk_copy(f, 0, ps)
            for tt in range(4):
                psv = qkps.tile([P, 512], F32, tag="qk")
                for c in range(DC):
                    nc.tensor.matmul(
                        psv, xs0[:, c, P * tt : P * (tt + 1)], wv_s[:, c, :],
                        start=(c == 0), stop=(c == DC - 1),
                    )
                nc.vector.tensor_copy(
                    vt[:, tt, :, 1 : DH + 1], psv.rearrange("p (h d) -> p h d", h=HPC)
                )

            # (steps, effective-iters for pacing, startup delay) — iters is set
            # ABOVE the call count so 12-20 fillers remain at each slab end:
            # the drain covers the boundary (next-slab warmup / last-pair
            # normalize wait) and keeps the PE clocked through it
            PACE = {0: (144, 26, 0), 1: (96, 44, 3), 2: (96, 60, 3), 3: (96, 90, 2)}
            xs_cur = xs0
            for j in range(NSLAB):
                xs_next = None
                if j + 1 < NSLAB:
                    xs_next = xpool.tile([P, DC, 512], BF16, tag="xs")
                    nc.gpsimd.dma_start(
                        xs_next, xt_d[:, :, 512 * (j + 1) : 512 * (j + 2)]
                    )
                gen = filler_gen(j, xs_cur, xs_next)
                steps, iters, delay = PACE[j]
                state = [-delay, 0]

                def fill():
                    state[0] += 1
                    if state[0] <= 0:
                        return
                    want = min(steps, steps * state[0] // iters) - state[1]
                    for _ in range(want):
                        if next(gen, "END") == "END":
                            break
                        state[1] += 1

                for hp in range(HPC // 2):
                    attn_pair(j, hp, fill)
                while next(gen, "END") != "END":
                    pass
                if xs_next is not None:
                    xs_cur = xs_next
            emit_proj_direct(NSLAB - 1)
        lp.__exit__(None, None, None)
    nc.compile()
    return nc


def _host_masks():
    m = np.zeros((P, 384), dtype=np.float32)
    for kl in range(P):
        m[kl, 0:128] = (np.arange(128) >= kl).astype(np.float32)       # triangle
        m[kl, 128:384] = (np.arange(256, 512) >= kl + 384).astype(np.float32)  # p=3 tail
    return m


def kernel(x, w_attn, w_proj):
    global _cached_nc, LAST_EXEC_NS
    x = np.asarray(x, dtype=np.float32)
    w_attn = np.asarray(w_attn, dtype=np.float32)
    w_proj = np.asarray(w_proj, dtype=np.float32)

    if _cached_nc is None:
        _cached_nc = _build_program()
    nc = _cached_nc

    masks = _host_masks().astype(NP_BF16)
    in_maps = []
    for c in range(N_CORES):
        b, hg = c // 2, (c % 2) * HPC
        w_q = w_attn[hg * DH : hg * DH + HPC * DH, :]
        w_k = w_attn[D + hg * DH : D + hg * DH + HPC * DH, :]
        w_v = w_attn[2 * D + hg * DH : 2 * D + hg * DH + HPC * DH, :]
        xt = x[b].T  # [D, T]
        wqk = np.concatenate([w_q, w_k], axis=0).T  # [D, 2*HPC*DH]
        wv = w_v.T  # [D, HPC*DH]
        wp = w_proj[:, hg * DH : hg * DH + HPC * DH].T  # [HPC*DH, D]
        in_maps.append(
            {
                # partition-major layouts: [p, c, ...] with d = c*128 + p
                "xt": np.ascontiguousarray(
                    xt.reshape(DC, P, T).transpose(1, 0, 2)
                ).astype(NP_BF16),
                "wqk": np.ascontiguousarray(
                    wqk.reshape(DC, P, 8, P).transpose(2, 1, 0, 3)
                ).astype(NP_BF16),
                "wv": np.ascontiguousarray(
                    wv.reshape(DC, P, HPC * DH).transpose(1, 0, 2)
                ).astype(NP_BF16),
                "wp": np.ascontiguousarray(
                    wp.reshape(HPC * DH // P, P, D).transpose(1, 0, 2)
                ),
                "masks": masks,
            }
        )

    res = run_bass_kernel_spmd(nc, in_maps, list(range(N_CORES)))
    LAST_EXEC_NS = res.exec_time_ns
    y = np.empty((B, T, D), dtype=np.float32)
    for b in range(B):
        y[b] = res.results[2 * b]["out"] + res.results[2 * b + 1]["out"]
    return y
